# revision 21
# baseline (speedup 1.0000x reference)
"""Trainium2 Bass kernel for the pointer-generator decoder step.

Contract: kernel(**inputs) takes the FULL unsharded inputs (as produced by the
problem's setup_inputs()) and returns the FULL [B, V+OOV] output.

Sharding (8 NeuronCores, one SPMD launch):
  * Front end (LSTM step, attention, context, p_gen, fc1) is data-parallel
    over batch (32 rows/core); encoder_outputs is read once machine-wide.
  * z1 = fc1 output is AllGathered (bf16, 0.5 MB) so every core holds the
    full batch.
  * fc2 (26 GFLOP) is tensor-parallel over vocab: each core computes
    exp(logits) for its 6250-column slice of the full batch plus the partial
    softmax denominators.  Denominator combination, gen-scaling, the OOV
    extension and the copy-mechanism scatter-add run on the host during
    output assembly (tiny per-element work vs. a 2nd on-chip collective).

Precision: attention path (h, e, softmax, att_copy) runs fp32/f32r; the
context einsum, fc1 and fc2 run bf16 (inputs are O(1), logits are O(0.1);
resulting output error ~1e-3 relative, dominated by p_gen rounding).
"""

import os
import sys

for _p in ("/opt/trn_rl_repo",):
    if _p not in sys.path and os.path.isdir(_p):
        sys.path.insert(0, _p)

import ml_dtypes
import numpy as np

import concourse.bass as bass
import concourse.bacc as bacc_mod
import concourse.mybir as mybir
import concourse.tile as tile
from concourse.bass_utils import run_bass_kernel_spmd
from concourse.masks import make_identity

NCORES = 8
B = 256           # batch
BC = B // NCORES  # batch shard per core (32)
I = 256           # input dim
H = 512           # hidden dim
A = 400           # attention dim
V = 50000         # vocab
VC = V // NCORES  # vocab shard per core (6250)
NT = 512          # vocab tile (psum bank) size

F32 = mybir.dt.float32
F32R = mybir.dt.float32r
BF16 = mybir.dt.bfloat16
AF = mybir.ActivationFunctionType
ALU = mybir.AluOpType
AX = mybir.AxisListType

# 400 split into PE-friendly chunks
CH4 = [(0, 128), (128, 128), (256, 128), (384, 16)]


def _bc(ap, parts):
    """Broadcast a DRAM AP across `parts` partitions (0-stride partition dim)."""
    return bass.AP(tensor=ap.tensor, offset=ap.offset, ap=[[0, parts]] + list(ap.ap))


def _vocab_tiles():
    out = []
    n0 = 0
    while n0 < VC:
        out.append((n0, min(NT, VC - n0)))
        n0 += NT
    return out


def build_nc(with_fc1_bias: bool) -> bass.Bass:
    nc = bacc_mod.Bacc("TRN2", target_bir_lowering=False, num_devices=NCORES)

    # ---- external inputs ----
    x0 = nc.dram_tensor("x0", [BC, I], F32, kind="ExternalInput")
    x0T = nc.dram_tensor("x0T", [I, BC], BF16, kind="ExternalInput")
    esT = nc.dram_tensor("esT", [H, BC], F32, kind="ExternalInput")
    enco = nc.dram_tensor("enco", [BC, 512, A], BF16, kind="ExternalInput")
    wihT = nc.dram_tensor("wihT", [I, 12 * 128], BF16, kind="ExternalInput")
    bg = nc.dram_tensor("bg", [12 * 128], F32, kind="ExternalInput")
    whsw = nc.dram_tensor("whsw", [H, 2 * A], F32, kind="ExternalInput")
    # packed: attb(400) | vvec(400) | pg2(400) | pg1(256) | pg3(512)
    smallp = nc.dram_tensor("smallp", [1968], F32, kind="ExternalInput")
    fc1a = nc.dram_tensor("fc1a", [A, 2 * H], BF16, kind="ExternalInput")
    fc1b = nc.dram_tensor("fc1b", [H, 2 * H], BF16, kind="ExternalInput")
    fc2wT = nc.dram_tensor("fc2wT", [2 * H, VC], BF16, kind="ExternalInput")
    if with_fc1_bias:
        fc1bias = nc.dram_tensor("fc1bias", [2 * H], F32, kind="ExternalInput")

    # ---- external outputs ----
    p_out = nc.dram_tensor("p_out", [B, VC], BF16, kind="ExternalOutput")
    attcopy_out = nc.dram_tensor("attcopy_out", [BC, A], F32, kind="ExternalOutput")
    s_out = nc.dram_tensor("s_out", [B, 1], F32, kind="ExternalOutput")
    gen_out = nc.dram_tensor("gen_out", [BC, 1], F32, kind="ExternalOutput")

    RG = [list(range(NCORES))]

    from contextlib import ExitStack

    with tile.TileContext(nc) as tc, ExitStack() as ctx:
        dram = ctx.enter_context(tc.tile_pool(name="dram", bufs=1, space="DRAM"))
        z1g_c = dram.tile([BC, 1024], BF16)
        z1g_full = dram.tile([B, 1024], BF16, addr_space="Shared")

        const = ctx.enter_context(tc.tile_pool(name="const", bufs=1))
        small = ctx.enter_context(tc.tile_pool(name="small", bufs=4))
        psA = ctx.enter_context(tc.tile_pool(name="psA", bufs=3, space="PSUM"))
        psT = ctx.enter_context(tc.tile_pool(name="psT", bufs=1, space="PSUM"))
        psC = ctx.enter_context(tc.tile_pool(name="psC", bufs=4, space="PSUM"))

        ident = const.tile([128, 128], F32)
        make_identity(nc, ident)
        ident_bf = const.tile([128, 128], BF16)
        make_identity(nc, ident_bf)

        # ---- constant loads ----
        wih_sb = const.tile([128, 2, 12 * 128], BF16)
        nc.sync.dma_start(out=wih_sb, in_=wihT[:].rearrange("(k p) m -> p k m", p=128))
        bg_sb = const.tile([128, 12], F32)
        nc.sync.dma_start(out=bg_sb, in_=bg[:].rearrange("(m p) -> p m", p=128))
        x0T_sb = const.tile([128, 2, BC], BF16)
        nc.sync.dma_start(out=x0T_sb, in_=x0T[:].rearrange("(k p) b -> p k b", p=128))
        x0_sb = const.tile([BC, I], F32)
        nc.gpsimd.dma_start(out=x0_sb, in_=x0[:])
        esT_sb = const.tile([128, 4, BC], F32R)
        nc.sync.dma_start(out=esT_sb, in_=esT[:].bitcast(F32R).rearrange("(k p) b -> p k b", p=128))
        whsw_sb = const.tile([128, 4, 2 * A], F32R)
        nc.sync.dma_start(out=whsw_sb, in_=whsw[:].bitcast(F32R).rearrange("(k p) a -> p k a", p=128))
        whwT_sb = whsw_sb[:, :, 0:A]
        wswT_sb = whsw_sb[:, :, A:2 * A]
        smallc = const.tile([BC, 1968], F32)
        nc.gpsimd.dma_start(out=smallc, in_=_bc(smallp[:], BC))
        attb_sb = smallc[:, 0:400]
        v_sb = smallc[:, 400:800]
        pg2_sb = smallc[:, 800:1200]
        pg1_sb = smallc[:, 1200:1456]
        pg3_sb = smallc[:, 1456:1968]
        # fc1 weights, rows chunked (128,128,128,16) to match ctxT chunks
        fc1a_sb = const.tile([128, 4, 2 * H], BF16)
        nc.sync.dma_start(
            out=fc1a_sb[:, 0:3, :],
            in_=fc1a[0:384, :].rearrange("(k p) n -> p k n", p=128),
        )
        nc.sync.dma_start(out=fc1a_sb[0:16, 3, :], in_=fc1a[384:400, :])
        fc1b_sb = const.tile([128, 4, 2 * H], BF16)
        nc.gpsimd.dma_start(out=fc1b_sb, in_=fc1b[:].rearrange("(k p) n -> p k n", p=128))
        if with_fc1_bias:
            fc1bias_sb = const.tile([BC, 2 * H], F32)
            nc.sync.dma_start(out=fc1bias_sb, in_=_bc(fc1bias[:], BC))

        # ---- LSTM step (h only; c0=h0=0 so the f-gate and W_hh are dead) ----
        sg_sb = const.tile([128, 12, BC], F32)  # sig(i), tanh(g), sig(o)
        for m in range(12):
            ps_g = psA.tile([128, BC], F32, tag="mmA")
            for k in range(2):
                nc.tensor.matmul(
                    out=ps_g,
                    lhsT=wih_sb[:, k, m * 128:(m + 1) * 128],
                    rhs=x0T_sb[:, k, :],
                    start=(k == 0),
                    stop=(k == 1),
                )
            func = AF.Tanh if 4 <= m < 8 else AF.Sigmoid
            nc.scalar.activation(
                out=sg_sb[:, m, :], in_=ps_g, func=func,
                bias=bg_sb[:, m:m + 1], scale=1.0,
            )
        cth_sb = const.tile([128, 4, BC], F32)  # tanh(c)
        nc.vector.tensor_mul(out=cth_sb, in0=sg_sb[:, 0:4, :], in1=sg_sb[:, 4:8, :])
        nc.scalar.activation(out=cth_sb, in_=cth_sb, func=AF.Tanh)
        hT_sb = const.tile([128, 4, BC], F32R)  # h feature-major (attention lhsT)
        nc.vector.tensor_mul(out=hT_sb, in0=sg_sb[:, 8:12, :], in1=cth_sb)
        hT_bf = const.tile([128, 4, BC], BF16)  # h feature-major (fc1 lhsT)
        nc.scalar.copy(out=hT_bf, in_=hT_sb.bitcast(F32))

        # h batch-major [32, 512] (for the p_gen dot)
        h_sb = const.tile([BC, H], F32)
        for k in range(4):
            ps_t = psT.tile([BC, 128], F32, tag="tp")
            nc.tensor.transpose(ps_t, hT_sb[:, k, :].bitcast(F32), ident)
            nc.scalar.copy(out=h_sb[:, k * 128:(k + 1) * 128], in_=ps_t)

        # ---- attention scores e = tanh(es @ WhwT + h @ WswT + attb)  [32,400]
        ps_e = psA.tile([BC, A], F32, tag="mmA")
        for k in range(4):
            nc.tensor.matmul(
                out=ps_e, lhsT=esT_sb[:, k, :], rhs=whwT_sb[:, k, :],
                start=(k == 0), stop=False,
            )
        for k in range(4):
            nc.tensor.matmul(
                out=ps_e, lhsT=hT_sb[:, k, :], rhs=wswT_sb[:, k, :],
                start=False, stop=(k == 3),
            )
        e_sb = const.tile([BC, A], F32)
        nc.vector.scalar_tensor_tensor(
            out=e_sb, in0=ps_e, scalar=1.0, in1=attb_sb,
            op0=ALU.mult, op1=ALU.add,
        )
        nc.scalar.activation(out=e_sb, in_=e_sb, func=AF.Tanh)

        # softmax over free dim, then * v  ->  att [32,400]
        mneg = small.tile([BC, 1], F32)
        nc.vector.tensor_reduce(out=mneg, in_=e_sb, axis=AX.X, op=ALU.max, negate=True)
        ssum = small.tile([BC, 1], F32)
        nc.scalar.activation(
            out=e_sb, in_=e_sb, func=AF.Exp, bias=mneg, scale=1.0, accum_out=ssum,
        )
        rs = small.tile([BC, 1], F32)
        nc.vector.reciprocal(out=rs, in_=ssum)
        att_sb = const.tile([BC, 512], F32)
        nc.vector.memset(att_sb[:, A:512], 0.0)
        nc.vector.scalar_tensor_tensor(
            out=att_sb[:, 0:A], in0=e_sb, scalar=rs, in1=v_sb,
            op0=ALU.mult, op1=ALU.mult,
        )

        # att feature-major (bf16) for the context matmuls, 4x128 over padded 512
        attT_sb = const.tile([128, 4, BC], BF16)
        for t in range(4):
            ps_t = psT.tile([128, BC], F32, tag="tp")
            nc.tensor.transpose(
                ps_t, att_sb[:, t * 128:(t + 1) * 128], ident[:BC, :BC]
            )
            nc.scalar.copy(out=attT_sb[:, t, :], in_=ps_t)

        # ---- context[b,:] = att[b] @ enco[b]: stationary att column, moving
        # enco rows (N=400).  Each row lands in psum [1,400]; rows are packed
        # into partition 0 of rows_sb, then one SBUF->SBUF DMA restores batch
        # layout.
        front_ctx = ExitStack()
        rowsp = front_ctx.enter_context(tc.tile_pool(name="rowsp", bufs=1))
        rows_sb = rowsp.tile([1, BC, A], F32)
        eop = front_ctx.enter_context(tc.tile_pool(name="eop", bufs=3))
        for b0 in range(0, BC, 4):
            eo_sb = eop.tile([128, 16, A], BF16, tag="eo")
            nc.sync.dma_start(
                out=eo_sb,
                in_=enco[b0:b0 + 4].rearrange("b (t p) e -> p (b t) e", p=128),
            )
            for bi in range(4):
                b = b0 + bi
                ps_row = psC.tile([1, A], F32, tag="psc")
                for t in range(4):
                    nc.tensor.matmul(
                        out=ps_row,
                        lhsT=attT_sb[:, t, b:b + 1],
                        rhs=eo_sb[:, bi * 4 + t, :],
                        start=(t == 0),
                        stop=(t == 3),
                    )
                if b % 2 == 0:
                    nc.scalar.copy(out=rows_sb[:, b, :], in_=ps_row)
                else:
                    nc.vector.tensor_copy(out=rows_sb[:, b, :], in_=ps_row)
        # context batch-major [32, 400]
        ctx_sb = const.tile([BC, A], F32)
        nc.sync.dma_start(out=ctx_sb, in_=rows_sb)
        front_ctx.close()
        # context feature-major (bf16) for fc1, chunks (128,128,128,16)
        ctxT_bf = const.tile([128, 4, BC], BF16)
        for t, (e0, esz) in enumerate(CH4):
            ps_t = psT.tile([128, BC], F32, tag="tp")
            nc.tensor.transpose(ps_t[:esz, :], ctx_sb[:, e0:e0 + esz], ident[:BC, :BC])
            nc.scalar.copy(out=ctxT_bf[:esz, t, :], in_=ps_t[:esz, :])

        # ---- p_gen = sigmoid(x0.pg1 + ctx.pg2 + h.pg3) ----
        dot_tmp = small.tile([BC, H], F32, tag="dtmp", bufs=2)
        acc1 = small.tile([BC, 1], F32)
        nc.vector.scalar_tensor_tensor(
            out=dot_tmp[:, :I], in0=x0_sb, scalar=1.0, in1=pg1_sb,
            op0=ALU.mult, op1=ALU.mult, accum_out=acc1,
        )
        dot_tmp2 = small.tile([BC, H], F32, tag="dtmp", bufs=2)
        acc2 = small.tile([BC, 1], F32)
        nc.vector.scalar_tensor_tensor(
            out=dot_tmp2[:, :A], in0=ctx_sb, scalar=1.0, in1=pg2_sb,
            op0=ALU.mult, op1=ALU.mult, accum_out=acc2,
        )
        dot_tmp3 = small.tile([BC, H], F32, tag="dtmp", bufs=2)
        acc3 = small.tile([BC, 1], F32)
        nc.vector.scalar_tensor_tensor(
            out=dot_tmp3, in0=h_sb, scalar=1.0, in1=pg3_sb,
            op0=ALU.mult, op1=ALU.mult, accum_out=acc3,
        )
        nc.vector.tensor_add(out=acc1, in0=acc1, in1=acc2)
        nc.vector.tensor_add(out=acc1, in0=acc1, in1=acc3)
        gen_sb = small.tile([BC, 1], F32)
        nc.scalar.activation(out=gen_sb, in_=acc1, func=AF.Sigmoid)
        nc.sync.dma_start(out=gen_out[:], in_=gen_sb)
        gen1m = small.tile([BC, 1], F32)
        nc.scalar.activation(out=gen1m, in_=gen_sb, func=AF.Identity, bias=1.0, scale=-1.0)

        # att_copy = (1-gen) * att  -> output
        attcopy_sb = const.tile([BC, A], F32)
        nc.vector.tensor_scalar_mul(out=attcopy_sb, in0=att_sb[:, 0:A], scalar1=gen1m)
        nc.sync.dma_start(out=attcopy_out[:], in_=attcopy_sb)

        # ---- fc1: z1 = [ctx | h] @ fc1_w^T (+ fc1_b), bf16 out [32, 1024] ----
        z1g_sb = const.tile([BC, 1024], BF16)
        for nh in range(2):
            ps_z = psA.tile([BC, NT], F32, tag="mmA")
            ns = slice(nh * NT, (nh + 1) * NT)
            for k, (c0, csz) in enumerate(CH4):
                nc.tensor.matmul(
                    out=ps_z, lhsT=ctxT_bf[:csz, k, :], rhs=fc1a_sb[:csz, k, ns],
                    start=(k == 0), stop=False,
                )
            for k in range(4):
                nc.tensor.matmul(
                    out=ps_z, lhsT=hT_bf[:, k, :], rhs=fc1b_sb[:, k, ns],
                    start=False, stop=(k == 3),
                )
            if with_fc1_bias:
                nc.vector.scalar_tensor_tensor(
                    out=z1g_sb[:, ns], in0=ps_z, scalar=1.0,
                    in1=fc1bias_sb[:, ns], op0=ALU.mult, op1=ALU.add,
                )
            else:
                nc.scalar.copy(out=z1g_sb[:, ns], in_=ps_z)
        nc.sync.dma_start(out=z1g_c[:], in_=z1g_sb)

        # ---- AllGather z1 across the 8 cores (bf16, 64KB -> 512KB) ----
        nc.gpsimd.collective_compute(
            "AllGather", ALU.bypass, replica_groups=RG,
            ins=[z1g_c.opt()], outs=[z1g_full.opt()],
        )

        # z1^T [128, 8, 256] bf16 via on-chip transposes
        z1T_sb = const.tile([128, 8, B], BF16)
        zrp = ctx.enter_context(tc.tile_pool(name="zrp", bufs=2))
        for bh in range(2):
            zrow = zrp.tile([128, 1024], BF16, tag="zrow")
            nc.sync.dma_start(out=zrow, in_=z1g_full[bh * 128:(bh + 1) * 128, :])
            for k2 in range(4):
                ps_t = psT.tile([128, 2, 128], BF16, tag="tp")
                for j in range(2):
                    nc.tensor.transpose(
                        ps_t[:, j, :], zrow[:, (k2 * 2 + j) * 128:(k2 * 2 + j + 1) * 128], ident_bf
                    )
                nc.scalar.copy(
                    out=z1T_sb[:, k2 * 2:k2 * 2 + 2, bh * 128:(bh + 1) * 128], in_=ps_t
                )

        # ---- fc2 (bf16): exp(logits) per vocab tile + partial denominators ----
        s_acc = small.tile([128, 2], F32, tag="sacc")
        nc.vector.memset(s_acc, 0.0)
        wp = ctx.enter_context(tc.tile_pool(name="wp", bufs=8))
        op_ = ctx.enter_context(tc.tile_pool(name="op", bufs=3))
        for n0, nt in _vocab_tiles():
            wn_sb = wp.tile([128, 8, NT], BF16, tag="wn")
            nc.sync.dma_start(
                out=wn_sb[:, 0:4, :nt],
                in_=fc2wT[0:512, n0:n0 + nt].rearrange("(k p) j -> p k j", p=128),
            )
            nc.sync.dma_start(
                out=wn_sb[:, 4:8, :nt],
                in_=fc2wT[512:1024, n0:n0 + nt].rearrange("(k p) j -> p k j", p=128),
            )
            for bh in range(2):
                ps_l = psA.tile([128, NT], F32, tag="mmA")
                for k in range(8):
                    nc.tensor.matmul(
                        out=ps_l[:, :nt],
                        lhsT=z1T_sb[:, k, bh * 128:(bh + 1) * 128],
                        rhs=wn_sb[:, k, :nt],
                        start=(k == 0), stop=(k == 7),
                    )
                o_sb = op_.tile([128, NT], BF16, tag="osb")
                st = small.tile([128, 1], F32, tag="st")
                nc.scalar.activation(
                    out=o_sb[:, :nt], in_=ps_l[:, :nt], func=AF.Exp, accum_out=st,
                )
                nc.vector.tensor_add(
                    out=s_acc[:, bh:bh + 1], in0=s_acc[:, bh:bh + 1], in1=st
                )
                nc.sync.dma_start(
                    out=p_out[bh * 128:(bh + 1) * 128, n0:n0 + nt],
                    in_=o_sb[:, :nt],
                )
        for bh in range(2):
            nc.sync.dma_start(
                out=s_out[bh * 128:(bh + 1) * 128, :], in_=s_acc[:, bh:bh + 1]
            )

    nc.compile()
    return nc


_NC_CACHE = {}


def _get_nc(with_fc1_bias: bool) -> bass.Bass:
    if with_fc1_bias not in _NC_CACHE:
        _NC_CACHE[with_fc1_bias] = build_nc(with_fc1_bias)
    return _NC_CACHE[with_fc1_bias]


RUN_KW = {}        # test.py can set e.g. {"trace": True}
LAST_RESULT = {}   # test.py reads exec_time_ns etc.


def make_in_maps(inputs: dict):
    f32 = lambda a: np.ascontiguousarray(np.asarray(a), dtype=np.float32)
    bf16 = ml_dtypes.bfloat16

    x = f32(inputs["x"])[:, 0, :]              # [B, I]
    enco = f32(inputs["encoder_outputs"])      # [B, A, A]
    es = f32(inputs["encoder_state"])          # [B, H]
    W_ih = f32(inputs["W_ih"])                 # [4H, I]
    b = f32(inputs["b_ih"]) + f32(inputs["b_hh"])  # [4H]
    Wh_w = f32(inputs["Wh_w"])                 # [A, H]
    Ws_w = f32(inputs["Ws_w"])
    attb = f32(inputs["Wh_b"]) + f32(inputs["Ws_b"])  # [A]
    vvec = f32(inputs["v"])                    # [A]
    fc1_w = f32(inputs["fc1_w"])               # [2H, H+A]
    fc1_b = f32(inputs["fc1_b"])               # [2H]
    fc2_w = f32(inputs["fc2_w"])               # [V, 2H]
    pg1 = f32(inputs["pg1_w"])[0]              # [I]
    pg2 = f32(inputs["pg2_w"])[0]              # [A]
    pg3 = f32(inputs["pg3_w"])[0]              # [H]

    with_fc1_bias = bool(np.any(fc1_b != 0.0))

    # i, g, o gate rows of W_ih / bias (f gate is dead: c0 = 0)
    idx = np.r_[0:H, 2 * H:3 * H, 3 * H:4 * H]
    wihT = np.ascontiguousarray(W_ih[idx].T).astype(bf16)   # [I, 1536]
    bg = np.ascontiguousarray(b[idx])          # [1536]

    whsw = np.concatenate([Wh_w.T, Ws_w.T], axis=1)  # [H, 2A]
    smallp = np.concatenate([attb, vvec, pg2, pg1, pg3])  # [1968]
    fc1T = fc1_w.T                              # [H+A, 2H]
    fc1a = np.ascontiguousarray(fc1T[:A]).astype(bf16)    # [A, 2H]
    fc1b_m = np.ascontiguousarray(fc1T[A:]).astype(bf16)  # [H, 2H]
    fc2T = np.ascontiguousarray(fc2_w.T).astype(bf16)     # [2H, V]
    enco_bf = np.zeros((B, 512, A), dtype=bf16)
    enco_bf[:, :A, :] = enco.astype(bf16)

    x0T = np.ascontiguousarray(x.T)             # [I, B]
    esT = np.ascontiguousarray(es.T)            # [H, B]

    in_maps = []
    for c in range(NCORES):
        bs = slice(c * BC, (c + 1) * BC)
        vs = slice(c * VC, (c + 1) * VC)
        m = {
            "x0": np.ascontiguousarray(x[bs]),
            "x0T": np.ascontiguousarray(x0T[:, bs]).astype(bf16),
            "esT": np.ascontiguousarray(esT[:, bs]),
            "enco": np.ascontiguousarray(enco_bf[bs]),
            "wihT": wihT,
            "bg": bg,
            "whsw": whsw,
            "smallp": smallp,
            "fc1a": fc1a,
            "fc1b": fc1b_m,
            "fc2wT": np.ascontiguousarray(fc2T[:, vs]),
        }
        if with_fc1_bias:
            m["fc1bias"] = fc1_b
        in_maps.append(m)
    return in_maps, with_fc1_bias


def kernel(**inputs) -> np.ndarray:
    in_maps, with_fc1_bias = make_in_maps(inputs)
    nc = _get_nc(with_fc1_bias)

    res = run_bass_kernel_spmd(nc, in_maps, core_ids=list(range(NCORES)), **RUN_KW)
    results = res.results
    LAST_RESULT["exec_time_ns"] = getattr(res, "exec_time_ns", None)

    oov = int(np.asarray(inputs["max_oov_nums"]))
    ids = np.asarray(inputs["ids"])
    fc2_b = np.asarray(inputs["fc2_b"], dtype=np.float32)

    gen = np.concatenate([np.asarray(results[c]["gen_out"])[:, 0] for c in range(NCORES)])
    p = np.zeros((B, V + oov), dtype=np.float32)
    if np.any(fc2_b != 0.0):
        # device computed exp(z); fold exp(fc2_b) in and recompute denominators
        g = np.exp(fc2_b.astype(np.float64)).astype(np.float32)
        for c in range(NCORES):
            vs = slice(c * VC, (c + 1) * VC)
            p[:, vs] = np.asarray(results[c]["p_out"], dtype=np.float32) * g[None, vs]
        s = p[:, :V].sum(axis=1)
        p[:, :V] *= (gen / s)[:, None]
    else:
        s = np.zeros(B, dtype=np.float32)
        for c in range(NCORES):
            s += np.asarray(results[c]["s_out"])[:, 0]
        f = (gen / s).astype(np.float32)
        for c in range(NCORES):
            vs = slice(c * VC, (c + 1) * VC)
            p[:, vs] = np.asarray(results[c]["p_out"], dtype=np.float32) * f[:, None]

    att_copy = np.concatenate(
        [np.asarray(results[c]["attcopy_out"]) for c in range(NCORES)], axis=0
    )
    rows = np.arange(B)[:, None]
    np.add.at(p, (rows, ids), att_copy)
    return p
